# revision 1
# baseline (speedup 1.0000x reference)
"""Single-head attention (B=8, S=2048, D=U=1024) on 8 TRN2 NeuronCores.

Sharding: data-parallel over batch — core b computes batch b end-to-end,
no cross-core communication.

Per-core pipeline (all matmuls bf16, fp32 PSUM accumulation):
  A. x [S,D] f32 --SWDGE cast--> DRAM bf16 staging blocks --xbar DMA
     transpose (sync HWDGE ring)--> xT [D,S] in SBUF.  The DRAM bounce
     exists because large xbar transposes need a DRAM source.
  B. W* f32 --SWDGE cast--> SBUF bf16 (half-width tiles, double buffered).
     SWDGE queue order (= emission order) is Wq.0, x blocks 0-3, Wq.1,
     Wk.0, Wk.1, Wv.0, Wv.1 — each arrives just before its consumer.
  C. Qt = (Wq^T xT + bq)/32  [U,S]   (lhsT=Wq, rhs=xT; bias+scale in epilogue)
     Kt = Wk^T xT + bk       [U,S]
     V  = xT^T Wv + bv       [S,U]   (lhsT=xT, rhs=Wv; bv broadcast-added in
     the DVE epilogue)
  D. scores^T[k,q] = sum_u Kt[u,k] Qt[u,q]; the padding mask adds the rank-1
     term c_k*m_q (c = -10000*(1-m)) via one DVE scalar_tensor_tensor per
     PSUM tile; Et = exp(scores^T) on ACT, PSUM->SBUF bf16.  No
     max-subtraction: scores are O(1) and masked entries underflow to
     exactly 0, matching the fp32 reference.
  E. ctx[q,u] = sum_k Et[k,q]^T V[k,u]  (lhsT=Et -> natural output layout);
     denom[q] via extra N=1 ones-column matmul under the same stationary Et;
     out = ctx * (1/denom) in the PSUM->SBUF epilogue (per-partition scalar).

SBUF: one long-lived pool; xT (phases A-C) and Et (D-E) share a 64KB tag
slot; small staging tiles and the E-phase output/reciprocal tiles reuse the
qt/kt/v tag slots outside those tensors' live ranges.
"""

import os
import sys

import numpy as np

for _p in ("/opt/trn_rl_repo", "/opt/pypackages"):
    if _p not in sys.path and os.path.isdir(_p):
        sys.path.append(_p)

import concourse.bass as bass
import concourse.tile as tile
from concourse import bacc, mybir
from concourse.bass import ts
from concourse.bass_utils import run_bass_kernel_spmd

P = 128
B, S, D, U = 8, 2048, 1024, 1024
NCORES = 8
NG = 512  # matmul moving free dim (one fp32 PSUM bank)
DT, UT, ST, KT = D // P, U // P, S // P, S // P  # 8, 8, 16, 16
SG, QG = S // NG, S // NG  # 4, 4
UG = U // NG  # 2
UH = UT // 2  # u-tiles per W half
SCALE = 1.0 / 32.0  # 1/sqrt(U)

F32 = mybir.dt.float32
BF16 = mybir.dt.bfloat16
I32 = mybir.dt.int32
AF = mybir.ActivationFunctionType
ALU = mybir.AluOpType

_cache = {}
last_results = None


def _emit(tc):
    nc = tc.nc
    x_d = nc.dram_tensor("x", [S, D], F32, kind="ExternalInput").ap()
    m_d = nc.dram_tensor("mask", [1, S], I32, kind="ExternalInput").ap()
    w_d = {
        "q": nc.dram_tensor("wq", [D, U], F32, kind="ExternalInput").ap(),
        "k": nc.dram_tensor("wk", [D, U], F32, kind="ExternalInput").ap(),
        "v": nc.dram_tensor("wv", [D, U], F32, kind="ExternalInput").ap(),
    }
    bq_d = nc.dram_tensor("bq", [1, U], F32, kind="ExternalInput").ap()
    bk_d = nc.dram_tensor("bk", [1, U], F32, kind="ExternalInput").ap()
    bv_d = nc.dram_tensor("bv", [1, U], F32, kind="ExternalInput").ap()
    out_d = nc.dram_tensor("out", [S, U], F32, kind="ExternalOutput").ap()

    # ---------------- small persistent tensors ----------------
    consts, free_consts = tc.tile(shape=[P, 2 * UT + KT], dtype=F32, name="consts")
    bq_cols = consts[:, 0:UT]
    bk_cols = consts[:, UT : 2 * UT]
    c_cols = consts[:, 2 * UT : 2 * UT + KT]  # -10000*(1-m), per k partition

    rows, free_rows = tc.tile(shape=[1, S + U + P], dtype=BF16, name="rows")
    m_row = rows[:, 0:S]
    bv_row = rows[:, S : S + U]
    ones_row = rows[:, S + U : S + U + P]

    ones_col, free_ones_col = tc.tile(shape=[P, 1], dtype=BF16, name="ones_col")
    m_bcast, free_m_bcast = tc.tile(shape=[P, S], dtype=BF16, name="m_bcast")
    bv_bcast, free_bv_bcast = tc.tile(shape=[P, U], dtype=BF16, name="bv_bcast")

    with tc.tile_pool(name="big", bufs=1) as big:

        def load_w_half(which, half):
            wt = big.tile([P, DT, NG], BF16, tag="w", bufs=2, name=f"w{which}_{half}")
            src = w_d[which].rearrange("(t p) u -> p t u", p=P)[:, :, ts(half, NG)]
            nc.gpsimd.dma_start(wt[:], src)  # f32 -> bf16 cast (SWDGE)
            return wt

        wq_h = [load_w_half("q", 0)]

        # small HWDGE loads up front (a few KB; must not trail the 32
        # transposes in the HWDGE queue)
        nc.sync.dma_start(bq_cols, bq_d.rearrange("a (j p) -> p (a j)", p=P))
        nc.sync.dma_start(bk_cols, bk_d.rearrange("a (j p) -> p (a j)", p=P))
        nc.vector.memset(ones_row, 1.0)
        nc.vector.memset(ones_col[:], 1.0)


        # ---------------- phase A: x -> bf16 -> transpose ----------------
        # slotA holds xT (A-C) then Et (D-E); sized for Et (64KB/partition).
        # SWDGE cast-DMAs stage bf16 x in DRAM; the xbar transposes
        # (serialized ~1.26us each on the sync ring) read it back per block.
        xT = big.tile([P, DT, S], BF16, tag="slotA", name="xT")
        SB = S // SG  # 512-row staging blocks
        with tc.tile_pool(name="xstage", bufs=SG, space="DRAM") as xstage:
            for sb in range(SG):
                blk = xstage.tile([SB, D], BF16, tag="xbf", name=f"xbf_{sb}")
                nc.gpsimd.dma_start(blk[:], x_d[ts(sb, SB), :])  # f32 -> bf16
                for dt in range(DT):
                    nc.sync.dma_start_transpose(xT[:, dt, ts(sb, SB)], blk[:, ts(dt, P)])
            wq_h.append(load_w_half("q", 1))

        # staging tiles ride the qt/kt/v tag slots, which are idle until C
        m_i32 = big.tile([1, S], I32, tag="qt", name="m_i32")
        nc.sync.dma_start(m_i32[:], m_d)
        nc.vector.tensor_copy(m_row, m_i32[:])
        mk_i32 = big.tile([P, KT], I32, tag="v", name="mk_i32")
        nc.sync.dma_start(mk_i32[:], m_d.rearrange("a (t p) -> p (a t)", p=P))
        # c = m*10000 - 10000  -> 0 where m==1, -10000 where m==0
        nc.vector.tensor_scalar(
            c_cols, mk_i32[:], 10000.0, -10000.0, ALU.mult, ALU.add
        )
        bv_f32 = big.tile([1, U], F32, tag="kt", name="bv_f32")
        nc.sync.dma_start(bv_f32[:], bv_d)
        nc.vector.tensor_copy(bv_row, bv_f32[:])

        # broadcast m and bv across partitions via ones-column matmuls
        with tc.tile_pool(name="psInit", bufs=2, space="PSUM") as psInit:
            for qg in range(QG):
                pi = psInit.tile([P, NG], F32, tag="init", name="ps_init")
                nc.tensor.matmul(
                    pi[:], lhsT=ones_row[:, 0:P], rhs=m_row[:, ts(qg, NG)]
                )
                nc.vector.tensor_copy(m_bcast[:, ts(qg, NG)], pi[:])
            for ug in range(UG):
                pi = psInit.tile([P, NG], F32, tag="init", name="ps_init2")
                nc.tensor.matmul(
                    pi[:], lhsT=ones_row[:, 0:P], rhs=bv_row[:, ts(ug, NG)]
                )
                nc.vector.tensor_copy(bv_bcast[:, ts(ug, NG)], pi[:])

        # ---------------- phase C: projections ----------------
        qt_sb = big.tile([P, UT, S], BF16, tag="qt", name="qt_sb")
        kt_sb = big.tile([P, UT, S], BF16, tag="kt", name="kt_sb")
        v_sb = big.tile([P, ST, U], BF16, tag="v", name="v_sb")

        with tc.tile_pool(name="psC", bufs=8, space="PSUM") as psC:
            # Q^T and K^T: [u,s] = sum_d W[d,u] * xT[d,s]
            for which, dst, bias_cols, scale in (
                ("q", qt_sb, bq_cols, SCALE),
                ("k", kt_sb, bk_cols, None),
            ):
                for half in range(2):
                    w_h = wq_h[half] if which == "q" else load_w_half(which, half)
                    for sg in range(SG):
                        for u4 in range(UH):
                            ut = half * UH + u4
                            ps = psC.tile([P, NG], F32, tag="proj", name="ps_proj")
                            for dt in range(DT):
                                nc.tensor.matmul(
                                    ps[:],
                                    lhsT=w_h[:, dt, ts(u4, P)],
                                    rhs=xT[:, dt, ts(sg, NG)],
                                    start=(dt == 0),
                                    stop=(dt == DT - 1),
                                )
                            if scale is not None:
                                nc.vector.tensor_scalar(
                                    dst[:, ut, ts(sg, NG)],
                                    ps[:],
                                    bias_cols[:, ut : ut + 1],
                                    scale,
                                    ALU.add,
                                    ALU.mult,
                                )
                            else:
                                nc.vector.tensor_scalar_add(
                                    dst[:, ut, ts(sg, NG)],
                                    ps[:],
                                    bias_cols[:, ut : ut + 1],
                                )

            # V: [s,u] = sum_d xT[d,s] * Wv[d,u]; bv added in the epilogue
            for ug in range(UG):
                wv_h = load_w_half("v", ug)
                for st in range(ST):
                    pv = psC.tile([P, NG], F32, tag="proj", name="ps_v")
                    for dt in range(DT):
                        nc.tensor.matmul(
                            pv[:],
                            lhsT=xT[:, dt, ts(st, P)],
                            rhs=wv_h[:, dt, :],
                            start=(dt == 0),
                            stop=(dt == DT - 1),
                        )
                    nc.vector.tensor_tensor(
                        v_sb[:, st, ts(ug, NG)],
                        pv[:],
                        bv_bcast[:, ts(ug, NG)],
                        ALU.add,
                    )

        # ---------------- phase D: scores^T + mask + exp ----------------
        et_sb = big.tile([P, KT, S], BF16, tag="slotA", name="et_sb")
        with tc.tile_pool(name="psD", bufs=6, space="PSUM") as psD:
            for kt in range(KT):
                pss = [
                    psD.tile([P, NG], F32, tag="sc", name="ps_sc") for _ in range(QG)
                ]
                for ut in range(UT):
                    for qg in range(QG):
                        nc.tensor.matmul(
                            pss[qg][:],
                            lhsT=kt_sb[:, ut, ts(kt, P)],
                            rhs=qt_sb[:, ut, ts(qg, NG)],
                            start=(ut == 0),
                            stop=(ut == UT - 1),
                        )
                for qg in range(QG):
                    # scores += c_k * m_q  (rank-1 mask term, on DVE)
                    nc.vector.scalar_tensor_tensor(
                        pss[qg][:],
                        m_bcast[:, ts(qg, NG)],
                        c_cols[:, kt : kt + 1],
                        pss[qg][:],
                        ALU.mult,
                        ALU.add,
                    )
                    nc.scalar.activation(et_sb[:, kt, ts(qg, NG)], pss[qg][:], AF.Exp)

        # ---------------- phase E: PV + denom + normalize ----------------
        with (
            tc.tile_pool(name="psE", bufs=4, space="PSUM") as psE,
            tc.tile_pool(name="psDen", bufs=2, space="PSUM") as psDen,
        ):
            for qt in range(KT):
                pc = [
                    psE.tile([P, NG], F32, tag="ctx", name="ps_ctx")
                    for _ in range(UG)
                ]
                den = psDen.tile([P, 1], F32, tag="den", name="ps_den")
                for kt in range(KT):
                    lhsT = et_sb[:, kt, ts(qt, P)]
                    first, last = kt == 0, kt == KT - 1
                    for ug in range(UG):
                        nc.tensor.matmul(
                            pc[ug][:],
                            lhsT=lhsT,
                            rhs=v_sb[:, kt, ts(ug, NG)],
                            start=first,
                            stop=last,
                        )
                    nc.tensor.matmul(
                        den[:], lhsT=lhsT, rhs=ones_col[:], start=first, stop=last
                    )
                recip = big.tile([P, 1], F32, tag="kt", name="recip")
                nc.vector.reciprocal(recip[:], den[:])
                o = big.tile([P, U], F32, tag="qt", name="o_sb")
                for ug in range(UG):
                    nc.vector.tensor_scalar_mul(o[:, ts(ug, NG)], pc[ug][:], recip[:])
                nc.sync.dma_start(out_d[ts(qt, P), :], o[:])

    free_bv_bcast()
    free_m_bcast()
    free_ones_col()
    free_rows()
    free_consts()


def _build():
    if "nc" in _cache:
        return _cache["nc"]
    nc = bacc.Bacc("TRN2", target_bir_lowering=False, debug=False, num_devices=NCORES)
    with tile.TileContext(nc) as tc:
        _emit(tc)
    nc.compile()
    _cache["nc"] = nc
    return nc


def kernel(x, mask, Wq, bq, Wk, bk, Wv, bv):
    global last_results
    nc = _build()
    wq = np.ascontiguousarray(Wq, dtype=np.float32)
    wk = np.ascontiguousarray(Wk, dtype=np.float32)
    wv = np.ascontiguousarray(Wv, dtype=np.float32)
    bqr = np.ascontiguousarray(bq, dtype=np.float32).reshape(1, U)
    bkr = np.ascontiguousarray(bk, dtype=np.float32).reshape(1, U)
    bvr = np.ascontiguousarray(bv, dtype=np.float32).reshape(1, U)
    in_maps = []
    for b in range(B):
        in_maps.append(
            {
                "x": np.ascontiguousarray(x[b], dtype=np.float32),
                "mask": np.ascontiguousarray(mask[b], dtype=np.int32).reshape(1, S),
                "wq": wq,
                "wk": wk,
                "wv": wv,
                "bq": bqr,
                "bk": bkr,
                "bv": bvr,
            }
        )
    res = run_bass_kernel_spmd(
        nc,
        in_maps,
        core_ids=list(range(NCORES)),
        trace=bool(int(os.environ.get("KERNEL_TRACE", "0"))),
        tmpdir=os.environ.get("KERNEL_TRACE_DIR"),
    )
    last_results = res
    return np.stack([res.results[b]["out"] for b in range(B)])



# revision 6
# speedup vs baseline: 1.2604x; 1.2604x over previous
"""Single-head attention (B=8, S=2048, D=U=1024) on 8 TRN2 NeuronCores.

Sharding: data-parallel over batch — core b computes batch b end-to-end,
no cross-core communication.

Per-core pipeline (fp32 PSUM accumulation everywhere):
  A. x [S,D] f32 --SWDGE cast--> x_sb [s,d] bf16 in SBUF (16 s-tiles),
     then PE-transposed 128x128 blocks (is_transpose matmul vs identity)
     -> PSUM -> DVE/ACT copies -> xT [D,S] bf16.  This replaces the old
     DRAM-bounce + 32 serialized xbar-DMA transposes (~40us of PE idle);
     the PE starts transposing as soon as the first s-tile lands and the
     first Q projection chunk is interleaved right behind it.
  B. W* f32 --SWDGE cast--> SBUF bf16 half-width tiles (3-slot ring).
     SWDGE queue order: Wq.0, x tiles 0-15, Wq.1, Wk.0, Wk.1, Wv.0, Wv.1
     — each arrives just before its consumer.
  C. Qt = Wq^T xT + bq  [U,S] fp8e4m3   (no 1/sqrt(U) here — folded into
     the exp scale in D so Q keeps fp8-friendly magnitude)
     Kt = Wk^T xT + bk  [U,S] fp8e4m3
     V  = xT^T Wv + bv  [S,U] bf16, with a ones column appended at u=1024
     (feeds the softmax denominator in E for free).
  D. scores^T[k,q] = sum_u Kt[u,k] Qt[u,q] via fp8 DoubleRow matmuls
     (2 u-tiles per instruction, 2x FLOP rate); the padding mask adds the
     rank-1 term c_k*m_q (c = -320000*(1-m), pre-scaled for the exp scale)
     via one DVE scalar_tensor_tensor per PSUM tile; Et = exp(scores^T/32)
     on ACT (scale=1/32), PSUM->SBUF bf16.  No max-subtraction: scores are
     O(1) and masked entries underflow to exactly 0, matching fp32 ref.
  E. ctx[q,u'] = sum_k Et[k,q]^T V[k,u']  over the 1025-wide V (u' = u plus
     the ones column) in a 342/342/341 column split (each <= one PSUM bank,
     no 1-column denominator matmuls); out = ctx * (1/denom) in the
     PSUM->SBUF epilogue (per-partition scalar), denom = ctx[:,1024].

Numerics: fp8 is used ONLY for the scores contraction (Q,K operands).
Simulated end-to-end rel err 1.80e-2 (threshold 2e-2); bf16 everywhere
else keeps the PV path at bf16 accuracy.  exp underflow handles masking
exactly; for m_q=0 rows softmax shift-invariance matches the reference.
"""

import os
import sys

import numpy as np

for _p in ("/opt/trn_rl_repo", "/opt/pypackages"):
    if _p not in sys.path and os.path.isdir(_p):
        sys.path.append(_p)

import concourse.bass as bass
import concourse.tile as tile
from concourse import bacc, mybir
from concourse.bass import ts
from concourse.bass_utils import run_bass_kernel_spmd
from concourse.masks import make_identity

P = 128
B, S, D, U = 8, 2048, 1024, 1024
NCORES = 8
NG = 512  # matmul moving free dim (one fp32 PSUM bank)
DT, UT, ST, KT = D // P, U // P, S // P, S // P  # 8, 8, 16, 16
SG, QG = S // NG, S // NG  # 4, 4
UG = U // NG  # 2
UH = UT // 2  # u-tiles per W half
VW = 1028  # v_sb row width: 1024 v cols + ones col at 1024 + pad to 8B
# phase-E 3-way split of the 1025 live v columns (each chunk <= 512)
ESPLIT = (342, 342, 341)

F32 = mybir.dt.float32
BF16 = mybir.dt.bfloat16
FP8 = mybir.dt.float8e4
I32 = mybir.dt.int32
AF = mybir.ActivationFunctionType
ALU = mybir.AluOpType
DR = mybir.MatmulPerfMode.DoubleRow

_cache = {}
last_results = None


def _emit(tc):
    nc = tc.nc
    x_d = nc.dram_tensor("x", [S, D], F32, kind="ExternalInput").ap()
    m_d = nc.dram_tensor("mask", [1, S], I32, kind="ExternalInput").ap()
    w_d = {
        "q": nc.dram_tensor("wq", [D, U], F32, kind="ExternalInput").ap(),
        "k": nc.dram_tensor("wk", [D, U], F32, kind="ExternalInput").ap(),
        "v": nc.dram_tensor("wv", [D, U], F32, kind="ExternalInput").ap(),
    }
    bq_d = nc.dram_tensor("bq", [1, U], F32, kind="ExternalInput").ap()
    bk_d = nc.dram_tensor("bk", [1, U], F32, kind="ExternalInput").ap()
    bv_d = nc.dram_tensor("bv", [1, U], F32, kind="ExternalInput").ap()
    out_d = nc.dram_tensor("out", [S, U], F32, kind="ExternalOutput").ap()

    # ---------------- small persistent tensors ----------------
    consts, free_consts = tc.tile(shape=[P, 2 * UT + KT], dtype=F32, name="consts")
    bq_cols = consts[:, 0:UT]
    bk_cols = consts[:, UT : 2 * UT]
    c_cols = consts[:, 2 * UT : 2 * UT + KT]  # -320000*(1-m), per k partition

    rows, free_rows = tc.tile(shape=[1, S + U + P], dtype=BF16, name="rows")
    m_row = rows[:, 0:S]
    bv_row = rows[:, S : S + U]
    ones_row = rows[:, S + U : S + U + P]

    ident, free_ident = tc.tile(shape=[P, P], dtype=BF16, name="ident")
    m_bcast, free_m_bcast = tc.tile(shape=[P, S], dtype=BF16, name="m_bcast")
    bv_bcast, free_bv_bcast = tc.tile(shape=[P, U], dtype=BF16, name="bv_bcast")

    make_identity(nc, ident[:])

    with tc.tile_pool(name="big", bufs=1) as big:

        def load_w_half(which, half, sliced=False):
            # sliced: one cast-DMA per 128-col u-slice so each proj_chunk
            # u4-iteration can start as soon as its own slice lands.
            wt = big.tile([P, DT, NG], BF16, tag="w", bufs=3, name=f"w{which}_{half}")
            src = w_d[which].rearrange("(t p) u -> p t u", p=P)[:, :, ts(half, NG)]
            if sliced:
                for u4 in range(UH):
                    nc.gpsimd.dma_start(wt[:, :, ts(u4, P)], src[:, :, ts(u4, P)])
            else:
                nc.gpsimd.dma_start(wt[:], src)  # f32 -> bf16 cast (SWDGE)
            return wt

        # x [s,d] -> SBUF bf16, one cast-DMA per 128-row s-tile (SWDGE).
        # Lives in the first 32KB of the 64KB slotA tag; et_sb takes the
        # slot over in phase D (after the last transpose read).
        # Queue order: x0 first (transposes are the first PE work), then
        # Wq.0 slices (C(q,0,0) consumes them u-slice by u-slice), then the
        # rest of x, then the remaining W halves just before their consumers.
        x_sb = big.tile([P, ST, D], BF16, tag="slotA", name="x_sb")
        x_src = x_d.rearrange("(t p) d -> p t d", p=P)

        def load_x(st):
            nc.gpsimd.dma_start(x_sb[:, st, :], x_src[:, st, :])

        load_x(0)
        wq_h = [load_w_half("q", 0, sliced=True)]
        for st in range(1, ST):
            load_x(st)
        wq_h.append(load_w_half("q", 1))

        # small HWDGE loads (a few KB, sync queue)
        m_i32 = big.tile([1, S], I32, tag="qt", name="m_i32")
        nc.sync.dma_start(m_i32[:], m_d)
        mk_i32 = big.tile([P, KT], I32, tag="v", name="mk_i32")
        nc.sync.dma_start(mk_i32[:], m_d.rearrange("a (t p) -> p (a t)", p=P))
        bv_f32 = big.tile([1, U], F32, tag="kt", name="bv_f32")
        nc.sync.dma_start(bv_f32[:], bv_d)
        nc.sync.dma_start(bq_cols, bq_d.rearrange("a (j p) -> p (a j)", p=P))
        nc.sync.dma_start(bk_cols, bk_d.rearrange("a (j p) -> p (a j)", p=P))

        nc.vector.memset(ones_row, 1.0)
        nc.vector.tensor_copy(m_row, m_i32[:])
        # c = m*320000 - 320000 -> 0 where m==1, -320000 where m==0
        # (exp applies scale=1/32, so this is -10000 in score units)
        nc.vector.tensor_scalar(
            c_cols, mk_i32[:], 320000.0, -320000.0, ALU.mult, ALU.add
        )
        nc.vector.tensor_copy(bv_row, bv_f32[:])

        # broadcast m and bv across partitions via ones-column matmuls
        with tc.tile_pool(name="psInit", bufs=2, space="PSUM") as psInit:
            for qg in range(QG):
                pi = psInit.tile([P, NG], F32, tag="init", name="ps_init")
                nc.tensor.matmul(
                    pi[:], lhsT=ones_row[:, 0:P], rhs=m_row[:, ts(qg, NG)]
                )
                nc.vector.tensor_copy(m_bcast[:, ts(qg, NG)], pi[:])
            for ug in range(UG):
                pi = psInit.tile([P, NG], F32, tag="init", name="ps_init2")
                nc.tensor.matmul(
                    pi[:], lhsT=ones_row[:, 0:P], rhs=bv_row[:, ts(ug, NG)]
                )
                nc.vector.tensor_copy(bv_bcast[:, ts(ug, NG)], pi[:])

        # ---------------- phases A+C interleaved ----------------
        xT = big.tile([P, DT, S], BF16, tag="xT", name="xT")
        qt_sb = big.tile([P, UT, S], FP8, tag="qt", name="qt_sb")
        kt_sb = big.tile([P, UT, S], FP8, tag="kt", name="kt_sb")
        v_sb = big.tile([P, ST, VW], BF16, tag="v", name="v_sb")
        nc.vector.memset(v_sb[:, :, U : U + 1], 1.0)  # denominator ones column

        with (
            tc.tile_pool(name="psT", bufs=2, space="PSUM") as psT,
            tc.tile_pool(name="psC", bufs=6, space="PSUM") as psC,
        ):
            copy_engines = (nc.vector, nc.scalar)

            def transpose_st(st):
                # x_sb[:, st, :] [s128, d1024] -> xT[:, :, st*128] [d, s128]
                pt = psT.tile([P, DT, P], BF16, tag="t", name="ps_t")
                for dt in range(DT):
                    nc.tensor.transpose(
                        pt[:, dt, :], x_sb[:, st, ts(dt, P)], ident[:]
                    )
                eng = copy_engines[st % 2]
                if eng is nc.scalar:
                    eng.copy(xT[:, :, ts(st, P)], pt[:])
                else:
                    eng.tensor_copy(xT[:, :, ts(st, P)], pt[:])

            def proj_chunk(which, half, sg):
                # Q^T / K^T: [u,s] = sum_d W[d,u] * xT[d,s]
                w_h = wq_h[half] if which == "q" else wk_h[half]
                dst = qt_sb if which == "q" else kt_sb
                bias_cols = bq_cols if which == "q" else bk_cols
                for u4 in range(UH):
                    ut = half * UH + u4
                    ps = psC.tile([P, NG], F32, tag="proj", name="ps_proj")
                    for dt in range(DT):
                        nc.tensor.matmul(
                            ps[:],
                            lhsT=w_h[:, dt, ts(u4, P)],
                            rhs=xT[:, dt, ts(sg, NG)],
                            start=(dt == 0),
                            stop=(dt == DT - 1),
                        )
                    nc.vector.tensor_scalar_add(
                        dst[:, ut, ts(sg, NG)], ps[:], bias_cols[:, ut : ut + 1]
                    )

            for st in range(4):
                transpose_st(st)
            proj_chunk("q", 0, 0)
            for st in range(4, 8):
                transpose_st(st)
            proj_chunk("q", 0, 1)
            for st in range(8, 12):
                transpose_st(st)
            proj_chunk("q", 0, 2)
            for st in range(12, 16):
                transpose_st(st)
            proj_chunk("q", 0, 3)
            for sg in range(SG):
                proj_chunk("q", 1, sg)
            wk_h = [load_w_half("k", 0), load_w_half("k", 1)]
            for half in range(2):
                for sg in range(SG):
                    proj_chunk("k", half, sg)

            # V: [s,u] = sum_d xT[d,s] * Wv[d,u]; bv added in the epilogue
            for ug in range(UG):
                wv_h = load_w_half("v", ug)
                for st in range(ST):
                    pv = psC.tile([P, NG], F32, tag="proj", name="ps_v")
                    for dt in range(DT):
                        nc.tensor.matmul(
                            pv[:],
                            lhsT=xT[:, dt, ts(st, P)],
                            rhs=wv_h[:, dt, :],
                            start=(dt == 0),
                            stop=(dt == DT - 1),
                        )
                    nc.vector.tensor_tensor(
                        v_sb[:, st, ts(ug, NG)],
                        pv[:],
                        bv_bcast[:, ts(ug, NG)],
                        ALU.add,
                    )

        # ---------------- phase D: scores^T + mask + exp ----------------
        # fp8 DoubleRow: each matmul contracts two u-tiles (256 rows).
        et_sb = big.tile([P, KT, S], BF16, tag="slotA", name="et_sb")
        with tc.tile_pool(name="psD", bufs=8, space="PSUM") as psD:
            for kt in range(KT):
                pss = [
                    psD.tile([P, NG], F32, tag="sc", name="ps_sc") for _ in range(QG)
                ]
                for t in range(UT // 2):
                    for qg in range(QG):
                        nc.tensor.matmul(
                            pss[qg][:],
                            lhsT=kt_sb[:, 2 * t : 2 * t + 2, ts(kt, P)],
                            rhs=qt_sb[:, 2 * t : 2 * t + 2, ts(qg, NG)],
                            start=(t == 0),
                            stop=(t == UT // 2 - 1),
                            perf_mode=DR,
                        )
                for qg in range(QG):
                    # scores += c_k * m_q  (rank-1 mask term, on DVE —
                    # GPSIMD cannot access PSUM)
                    nc.vector.scalar_tensor_tensor(
                        pss[qg][:],
                        m_bcast[:, ts(qg, NG)],
                        c_cols[:, kt : kt + 1],
                        pss[qg][:],
                        ALU.mult,
                        ALU.add,
                    )
                    # Et = exp(scores/32); 1/sqrt(U) folded in here
                    nc.scalar.activation(
                        et_sb[:, kt, ts(qg, NG)], pss[qg][:], AF.Exp, scale=1.0 / 32.0
                    )

        # ---------------- phase E: PV(+denom column) + normalize ----------------
        e_off = (0, ESPLIT[0], ESPLIT[0] + ESPLIT[1])
        with tc.tile_pool(name="psE", bufs=6, space="PSUM") as psE:
            for qt in range(KT):
                pc = [
                    psE.tile([P, NG], F32, tag="ctx", name="ps_ctx") for _ in range(3)
                ]
                for kt in range(KT):
                    lhsT = et_sb[:, kt, ts(qt, P)]
                    first, last = kt == 0, kt == KT - 1
                    for j in range(3):
                        nc.tensor.matmul(
                            pc[j][:, 0 : ESPLIT[j]],
                            lhsT=lhsT,
                            rhs=v_sb[:, kt, e_off[j] : e_off[j] + ESPLIT[j]],
                            start=first,
                            stop=last,
                        )
                recip = big.tile([P, 1], F32, tag="kt", name="recip")
                # denominator = ones-column result: last col of chunk 2
                nc.vector.reciprocal(recip[:], pc[2][:, ESPLIT[2] - 1 : ESPLIT[2]])
                # per-chunk normalize + store so the final store starts as
                # early as possible (trims the kernel tail)
                o = big.tile([P, U], F32, tag="qt", name="o_sb")
                for j, w in ((0, ESPLIT[0]), (1, ESPLIT[1]), (2, ESPLIT[2] - 1)):
                    lo = e_off[j]
                    nc.vector.tensor_scalar_mul(
                        o[:, lo : lo + w], pc[j][:, 0:w], recip[:]
                    )
                    nc.sync.dma_start(
                        out_d[ts(qt, P), lo : lo + w], o[:, lo : lo + w]
                    )

    free_bv_bcast()
    free_m_bcast()
    free_ident()
    free_rows()
    free_consts()


def _build():
    if "nc" in _cache:
        return _cache["nc"]
    nc = bacc.Bacc("TRN2", target_bir_lowering=False, debug=False, num_devices=NCORES)
    with tile.TileContext(nc) as tc:
        _emit(tc)
    nc.compile()
    _cache["nc"] = nc
    return nc


def kernel(x, mask, Wq, bq, Wk, bk, Wv, bv):
    global last_results
    nc = _build()
    wq = np.ascontiguousarray(Wq, dtype=np.float32)
    wk = np.ascontiguousarray(Wk, dtype=np.float32)
    wv = np.ascontiguousarray(Wv, dtype=np.float32)
    bqr = np.ascontiguousarray(bq, dtype=np.float32).reshape(1, U)
    bkr = np.ascontiguousarray(bk, dtype=np.float32).reshape(1, U)
    bvr = np.ascontiguousarray(bv, dtype=np.float32).reshape(1, U)
    in_maps = []
    for b in range(B):
        in_maps.append(
            {
                "x": np.ascontiguousarray(x[b], dtype=np.float32),
                "mask": np.ascontiguousarray(mask[b], dtype=np.int32).reshape(1, S),
                "wq": wq,
                "wk": wk,
                "wv": wv,
                "bq": bqr,
                "bk": bkr,
                "bv": bvr,
            }
        )
    res = run_bass_kernel_spmd(
        nc,
        in_maps,
        core_ids=list(range(NCORES)),
        trace=bool(int(os.environ.get("KERNEL_TRACE", "0"))),
        tmpdir=os.environ.get("KERNEL_TRACE_DIR"),
    )
    last_results = res
    return np.stack([res.results[b]["out"] for b in range(B)])
